# revision 20
# baseline (speedup 1.0000x reference)
"""AttnBlock3D Trainium2 Bass kernel (8 NeuronCores, SPMD) — v2b (final).

Layout / algorithm (per core r, heads n = 2r, 2r+1):
  x viewed as [128=(t,c), 4096=hw].  BN stats pipelined behind the x DMA
  (per-chunk DVE reduce_sum + ACT Square-with-accum), combined over t per
  channel via selection matmuls.  gamma/beta, conv biases and the T^-0.5
  score scale are folded on the host into the projection weights.
  q & k are produced by ONE block-diag matmul (wqk_bd [128,128]: cols 0-63
  q-heads, 64-127 k-heads), evacuated psum->sbuf by DVE (q, +bias) and ACT
  (k, Copy+bias) in parallel.  v: 8 pixel-chunks per psum tile -> strided
  copies into vT9 [hw, 9] per head (col 0 = ones; accumulates sum(exp)).
  Attention per head: 4 i-windows of 1024; for each of 32 j-tiles: QK
  matmuls -> psum fp32 [128(j), 1024(i)] (3-deep rotation so the PE never
  stalls on the exp engines).  exp is split across BOTH engines: ACT does
  head0's tile (+ first EXSPLIT cols of head1) with the exact Exp LUT;
  DVE does the rest of head1 via a Schraudolph bit-trick (one
  tensor_scalar mult+add writing round-to-nearest int16 that IS the bf16
  bit pattern of exp(s); constant-factor bias cancels in softmax).
  AV matmuls (one j-tile behind) accumulate [9,512] blocks at psum
  partitions 32g over all 32 j-tiles.
  Unnormalized outputs + sumexp go through ONE AllToAll (each core
  receives only its own 512-column slice: 288KB instead of 2.3MB), then a
  per-core sliced tail: recip (fast approx) -> f-broadcast via DRAM
  bounce -> multiply -> block-diag wp matmul -> fused +bias+residual.
  Each core outputs its [128, 512] column slice; host concatenates.
"""
import sys

import numpy as np

sys.path.insert(0, "/opt/trn_rl_repo")

T, C, HW, NCORES = 8, 16, 4096, 8
N_ELEM = T * HW  # per-channel element count for BN stats
EPS = 1e-5
SCALE = float(T) ** -0.5
WWIN = 1024                     # i-window width
NWIN = HW // WWIN               # 4 windows
NJT = 32                        # j-tiles of 128
CHK = 1024                      # x load / stats chunk
EXSPLIT = 0                     # cols of head1's exp done by ACT (balance)
# Schraudolph bf16 exp: bits = trunc(s * A + B); bitcast int16 -> bf16.
# +0.5 turns the fp32->int16 truncation into round-to-nearest; c tuned to
# center the sawtooth rel-error (any constant factor cancels in softmax).
EXP_A = 128.0 / float(np.log(2.0))
EXP_C = 11.75
EXP_B = 127.0 * 128.0 - EXP_C + 0.5

_CACHE = {}


def _build_program():
    import concourse.bass as bass
    import concourse.bacc as bacc
    import concourse.tile as tile
    from concourse import mybir

    f32 = mybir.dt.float32
    bf16 = mybir.dt.bfloat16
    i16 = mybir.dt.int16
    AX = mybir.AxisListType
    OP = mybir.AluOpType
    ACT = mybir.ActivationFunctionType

    nc = bacc.Bacc("TRN2", target_bir_lowering=False, debug=False,
                   num_devices=NCORES)
    x = nc.dram_tensor("x", [128, HW], f32, kind="ExternalInput").ap()
    x_res = nc.dram_tensor("x_res", [128, 512], f32,
                           kind="ExternalInput").ap()
    wqk_bd = nc.dram_tensor("wqk_bd", [128, 128], bf16,
                            kind="ExternalInput").ap()
    bqk_col = nc.dram_tensor("bqk_col", [128, 1], f32,
                             kind="ExternalInput").ap()
    wv_rhs = nc.dram_tensor("wv_rhs", [128, 18], bf16,
                            kind="ExternalInput").ap()
    wp_bd = nc.dram_tensor("wp_bd", [128, 128], bf16,
                           kind="ExternalInput").ap()
    bp_col = nc.dram_tensor("bp_col", [128, 1], f32,
                            kind="ExternalInput").ap()
    sel = nc.dram_tensor("sel", [128, 16], f32, kind="ExternalInput").ap()
    out = nc.dram_tensor("out", [128, 512], f32, kind="ExternalOutput").ap()

    with tile.TileContext(nc) as tc:
        with (
            tc.tile_pool(name="persist", bufs=1) as P1,
            tc.tile_pool(name="work", bufs=4) as PW,
            tc.tile_pool(name="scratch", bufs=1) as PS,
            tc.tile_pool(name="psq", bufs=3, space="PSUM") as PQK,
            tc.tile_pool(name="psa", bufs=2, space="PSUM") as PAV,
            tc.tile_pool(name="dram", bufs=1, space="DRAM") as PD,
        ):
            # ---------------- loads ----------------
            x_sb = P1.tile([128, HW], f32)
            for ch in range(HW // CHK):
                nc.sync.dma_start(out=x_sb[:, ch * CHK:(ch + 1) * CHK],
                                  in_=x[:, ch * CHK:(ch + 1) * CHK])
            xres_sb = P1.tile([128, 512], f32)
            nc.sync.dma_start(out=xres_sb, in_=x_res)
            wqk_sb = P1.tile([128, 128], bf16)
            nc.gpsimd.dma_start(out=wqk_sb, in_=wqk_bd)
            bqk_sb = P1.tile([128, 1], f32)
            nc.gpsimd.dma_start(out=bqk_sb, in_=bqk_col)
            wvrhs_sb = P1.tile([128, 18], bf16)
            nc.gpsimd.dma_start(out=wvrhs_sb, in_=wv_rhs)
            wpbd_sb = P1.tile([128, 128], bf16)
            nc.gpsimd.dma_start(out=wpbd_sb, in_=wp_bd)
            bpcol_sb = P1.tile([128, 1], f32)
            nc.gpsimd.dma_start(out=bpcol_sb, in_=bp_col)
            sel_sb = P1.tile([128, 16], f32)
            nc.gpsimd.dma_start(out=sel_sb, in_=sel)

            # ---------------- BN stats (pipelined per chunk) -------------
            zero128 = P1.tile([128, 1], f32)
            nc.vector.memset(zero128, 0.0)
            eps_t = P1.tile([1, 1], f32)
            nc.vector.memset(eps_t, EPS)
            zero_t = P1.tile([1, 1], f32)
            nc.vector.memset(zero_t, 0.0)
            s1c = P1.tile([128, 4], f32)
            s2c = P1.tile([128, 4], f32)
            sqj = PS.tile([128, CHK], bf16, tag="sqj")
            for ch in range(HW // CHK):
                xc = x_sb[:, ch * CHK:(ch + 1) * CHK]
                nc.vector.reduce_sum(out=s1c[:, ch:ch + 1], in_=xc, axis=AX.X)
                nc.scalar.activation(sqj, xc, ACT.Square, bias=zero128,
                                     accum_out=s2c[:, ch:ch + 1])
            s1 = P1.tile([128, 2], f32)
            nc.vector.reduce_sum(out=s1[:, 0:1], in_=s1c, axis=AX.X)
            nc.vector.reduce_sum(out=s1[:, 1:2], in_=s2c, axis=AX.X)
            ps_st = PAV.tile([1, 32], f32, tag="av")
            nc.tensor.matmul(ps_st[:, 0:16], lhsT=s1[:, 0:1], rhs=sel_sb,
                             start=True, stop=True)
            nc.tensor.matmul(ps_st[:, 16:32], lhsT=s1[:, 1:2], rhs=sel_sb,
                             start=True, stop=True)
            stats = P1.tile([1, 32], f32)
            nc.vector.tensor_scalar_mul(stats, ps_st, 1.0 / N_ELEM)
            var = P1.tile([1, 16], f32)
            nc.vector.tensor_mul(var, stats[:, 0:16], stats[:, 0:16])
            nc.vector.tensor_sub(var, stats[:, 16:32], var)
            inv = P1.tile([1, 16], f32)
            nc.scalar.activation(inv, var, ACT.Ln, bias=eps_t)
            nc.scalar.activation(inv, inv, ACT.Exp, scale=-0.5, bias=zero_t)
            # bounce mean/inv through DRAM to broadcast [1,16] -> [128,1]
            st_dram = PD.tile([2, 16], f32)
            nc.sync.dma_start(out=st_dram[0:1, :], in_=stats[:, 0:16])
            nc.sync.dma_start(out=st_dram[1:2, :], in_=inv)
            mean_p = P1.tile([128, 1], f32)
            inv_p = P1.tile([128, 1], f32)
            for dst, row in ((mean_p, st_dram[0:1, :]),
                             (inv_p, st_dram[1:2, :])):
                src = bass.AP(tensor=row.tensor, offset=row.offset,
                              ap=[[0, T], list(row.ap[-1])])
                nc.gpsimd.dma_start(out=dst[:], in_=src)

            # ---------------- xhat (2 halves) ----------------------------
            xhat = P1.tile([128, HW], bf16)
            for h in range(2):
                sl = slice(h * 2048, (h + 1) * 2048)
                nc.vector.tensor_scalar(out=xhat[:, sl], in0=x_sb[:, sl],
                                        scalar1=mean_p, scalar2=inv_p,
                                        op0=OP.subtract, op1=OP.mult)

            # ---------------- q & k projection (fused, bf16) -------------
            q_sb = P1.tile([64, HW], bf16)
            k_sb = P1.tile([64, HW], bf16)
            for ch in range(HW // 1024):
                pp = PQK.tile([128, 1024], f32, tag="qk", name=f"proj{ch}")
                for b in range(2):
                    nc.tensor.matmul(pp[:, b * 512:(b + 1) * 512],
                                     lhsT=wqk_sb,
                                     rhs=xhat[:, ch * 1024 + b * 512:
                                              ch * 1024 + (b + 1) * 512],
                                     start=True, stop=True)
                csl = slice(ch * 1024, (ch + 1) * 1024)
                nc.vector.tensor_scalar_add(out=q_sb[:, csl],
                                            in0=pp[0:64, :],
                                            scalar1=bqk_sb[0:64])
                nc.scalar.activation(k_sb[:, csl], pp[64:128, :],
                                     ACT.Identity, bias=bqk_sb[64:128])

            # ---------------- v -> vT9 per head (bf16, ones col 0) -------
            vT9 = []
            for l in range(2):
                t9 = P1.tile([128, 32, 9], bf16, tag=f"t9_{l}")
                nc.vector.memset(t9[:, :, 0:1], 1.0)
                vT9.append(t9)
            for g in range(4):
                vps = PAV.tile([128, 144], f32, tag="av", name=f"vps{g}")
                for jj in range(8):
                    jc = g * 8 + jj
                    nc.tensor.matmul(vps[:, jj * 18:(jj + 1) * 18],
                                     lhsT=xhat[:, jc * 128:(jc + 1) * 128],
                                     rhs=wvrhs_sb, start=True, stop=True)
                vv = vps[:]
                for l in range(2):
                    src = bass.AP(tensor=vv.tensor,
                                  offset=vv.offset + l * 9 + 1,
                                  ap=[list(vv.ap[0]), [18, 8], [1, 8]])
                    dst = vT9[l][:, g * 8:(g + 1) * 8, 1:9]
                    if l == 0:
                        nc.scalar.activation(dst, src, ACT.Copy)
                    else:
                        nc.vector.tensor_copy(dst, src)

            # ---------------- attention ----------------
            cc_in = nc.dram_tensor("cc_in", [NCORES, 18 * 512], f32).ap()
            for w in range(NWIN):
                i0 = w * WWIN
                avs = [PAV.tile([128, 512], f32, tag="av",
                                name=f"av_{l}_{w}") for l in range(2)]
                # AV runs one j-tile behind QK/exp.
                ex_prev = [None, None]
                for jt in range(NJT + 1):
                    ex_cur = [None, None]
                    if jt < NJT:
                        qks = [PQK.tile([128, WWIN], f32, tag="qk",
                                        name=f"qk_{l}") for l in range(2)]
                        for b in range(WWIN // 512):
                            for l in range(2):
                                nc.tensor.matmul(
                                    qks[l][:, b * 512:(b + 1) * 512],
                                    lhsT=k_sb[l * 32:l * 32 + 8,
                                              jt * 128:(jt + 1) * 128],
                                    rhs=q_sb[l * 32:l * 32 + 8,
                                             i0 + b * 512:
                                             i0 + (b + 1) * 512],
                                    start=True, stop=True)
                        # exp: ACT head0 (+ first EXSPLIT of head1, exact),
                        # DVE rest of head1 (Schraudolph int16 == bf16 bits)
                        ex0 = PW.tile([128, WWIN], bf16, tag="ex")
                        nc.scalar.activation(ex0, qks[0], ACT.Exp, bias=zero128)
                        ex1 = PW.tile([128, WWIN], bf16, tag="ex")
                        nc.vector.tensor_scalar(
                            out=ex1[:].bitcast(i16),
                            in0=qks[1][:],
                            scalar1=float(EXP_A), scalar2=float(EXP_B),
                            op0=OP.mult, op1=OP.add)
                        ex_cur = [ex0, ex1]
                    if jt > 0:
                        for l in range(2):
                            for g in range(WWIN // 512):
                                nc.tensor.matmul(
                                    avs[l][32 * g:32 * g + 9, :],
                                    lhsT=vT9[l][:, jt - 1, :],
                                    rhs=ex_prev[l][:, g * 512:(g + 1) * 512],
                                    start=(jt == 1), stop=(jt == NJT),
                                    tile_position=(0, 32 * g),
                                    skip_group_check=True)
                    ex_prev = ex_cur
                # copy psum -> sbuf, ship unnormalized rows + sumexp to the
                # peer owning each 512-column chunk (AllToAll layout).
                for l in range(2):
                    s128 = PW.tile([128, 512], f32, tag="s128")
                    if l == 0:
                        nc.scalar.activation(s128, avs[l], ACT.Copy)
                    else:
                        nc.vector.tensor_copy(s128, avs[l])
                    for g in range(WWIN // 512):
                        p = 2 * w + g
                        dst = bass.AP(tensor=cc_in.tensor,
                                      offset=p * 9216 + l * 4608,
                                      ap=[[512, 9], [1, 512]])
                        nc.sync.dma_start(out=dst,
                                          in_=s128[32 * g:32 * g + 9, :])

            # ------------- AllToAll + sliced normalize + output proj -----
            cc_out = nc.dram_tensor("cc_out", [NCORES, 18 * 512], f32).ap()
            nc.gpsimd.collective_compute(
                "AllToAll", OP.bypass,
                replica_groups=[list(range(NCORES))],
                ins=[cc_in.opt()], outs=[cc_out.opt()])
            # sumexp rows of all 16 heads for MY column slice -> recip
            rsum = P1.tile([16, 512], f32)
            src = bass.AP(tensor=cc_out.tensor, offset=0,
                          ap=[[9216, 8], [4608, 2], [1, 512]])
            nc.sync.dma_start(out=rsum[:], in_=src)
            rinv = P1.tile([16, 512], f32)
            nc.vector.reciprocal_approx_fast(rinv, rsum)
            rdram = PD.tile([16, 512], f32)
            nc.sync.dma_start(out=rdram[:], in_=rinv[:])
            rbc = P1.tile([128, 512], f32)
            rd_t = rdram[:].tensor
            src2 = bass.AP(tensor=rd_t, offset=rdram[:].offset,
                           ap=[[512, 16], [0, T], [1, 512]])
            nc.sync.dma_start(out=rbc[:], in_=src2)
            # unnormalized attention rows (c,f layout) for MY columns
            acf = P1.tile([128, 512], f32)
            src3 = bass.AP(tensor=cc_out.tensor, offset=512,
                           ap=[[9216, 8], [4608, 2], [512, 8], [1, 512]])
            nc.sync.dma_start(out=acf[:], in_=src3)
            att_n = P1.tile([128, 512], bf16)
            nc.vector.tensor_mul(att_n, acf, rbc)
            psp = PQK.tile([128, 512], f32, tag="qk", name="pconv")
            nc.tensor.matmul(psp, lhsT=wpbd_sb, rhs=att_n,
                             start=True, stop=True)
            och = P1.tile([128, 512], f32)
            nc.vector.scalar_tensor_tensor(
                out=och, in0=psp, scalar=bpcol_sb,
                in1=xres_sb, op0=OP.add, op1=OP.add)
            nc.sync.dma_start(out=out, in_=och)

    nc.compile()
    return nc


def host_inputs(r, x128, gamma, beta, wq, bq, wk, bk, wv, bv, wp, bp):
    """Per-core host-side input prep (folds gamma/beta/biases/scale)."""
    import ml_dtypes
    bf = ml_dtypes.bfloat16
    wq_e = (wq * gamma[None, :] * SCALE).astype(np.float32)
    wk_e = (wk * gamma[None, :]).astype(np.float32)
    wv_e = (wv * gamma[None, :]).astype(np.float32)
    bq_e = ((bq + wq @ beta) * SCALE).astype(np.float32)
    bk_e = (bk + wk @ beta).astype(np.float32)
    bv_e = (bv + wv @ beta).astype(np.float32)
    bp_e = (bp + wp @ bv_e).astype(np.float32)

    wqk_bd = np.zeros((128, 128), np.float32)
    bqk_col = np.zeros((128, 1), np.float32)
    wv_rhs = np.zeros((128, 18), np.float32)
    fi = np.arange(T)
    ci = np.arange(C)
    for l in range(2):
        n = 2 * r + l
        rows = fi[:, None] * 16 + ci[None, :]
        wqk_bd[rows, (l * 32 + fi)[:, None]] = wq_e[n]
        wqk_bd[rows, (64 + l * 32 + fi)[:, None]] = wk_e[n]
        wv_rhs[rows, (l * 9 + 1 + fi)[:, None]] = wv_e[n]
        bqk_col[l * 32 + fi, 0] = bq_e[n]
        bqk_col[64 + l * 32 + fi, 0] = bk_e[n]
    # p-conv lhsT rows are in (c,f) order to match the gathered layout
    wp_bd = np.zeros((128, 128), np.float32)
    bp_col = np.zeros((128, 1), np.float32)
    for f in range(T):
        wp_bd[np.ix_(ci * 8 + f, f * 16 + ci)] = wp.T
        bp_col[f * 16 + ci, 0] = bp_e
    selm = np.zeros((128, 16), np.float32)
    selm[np.arange(128), np.tile(ci, T)] = 1.0
    x_res = np.ascontiguousarray(x128[:, r * 512:(r + 1) * 512])
    return dict(x=x128, x_res=x_res, wqk_bd=wqk_bd.astype(bf),
                bqk_col=bqk_col, wv_rhs=wv_rhs.astype(bf),
                wp_bd=wp_bd.astype(bf), bp_col=bp_col, sel=selm)


def make_in_maps(inputs):
    x = np.ascontiguousarray(np.asarray(inputs["x"], np.float32))
    x128 = x.reshape(128, HW)
    args = {k: np.asarray(v, np.float32) for k, v in inputs.items()
            if k != "x"}
    return [host_inputs(r, x128, **args) for r in range(NCORES)]


def run(inputs, trace=False):
    """Returns (out (8,16,64,64) f32, BassKernelResults)."""
    from concourse.bass_utils import run_bass_kernel_spmd
    if "nc" not in _CACHE:
        _CACHE["nc"] = _build_program()
    nc = _CACHE["nc"]
    in_maps = make_in_maps(inputs)
    res = run_bass_kernel_spmd(nc, in_maps, list(range(NCORES)), trace=trace)
    out128 = np.empty((128, HW), np.float32)
    for r in range(NCORES):
        out128[:, r * 512:(r + 1) * 512] = np.asarray(
            res.results[r]["out"], np.float32)
    return out128.reshape(T, C, 64, 64), res


def kernel(**inputs):
    out, _ = run(inputs, trace=False)
    return out


# revision 21
# speedup vs baseline: 1.2062x; 1.2062x over previous
"""AttnBlock3D Trainium2 Bass kernel (8 NeuronCores, SPMD) — v2.

Layout / algorithm (per core r, heads n = 2r, 2r+1):
  x viewed as [128=(t,c), 4096=hw].  BN stats pipelined behind the x DMA
  (per-chunk DVE reduce_sum + ACT Square-with-accum), combined over t per
  channel via selection matmuls.  gamma/beta, conv biases and the T^-0.5
  score scale are folded on the host into the projection weights.
  q & k are produced by ONE block-diag matmul (wqk_bd [128,128]: cols 0-63
  q-heads, 64-127 k-heads), evacuated psum->sbuf by DVE (q, +bias) and ACT
  (k, Copy+bias) in parallel.  v: 8 pixel-chunks per psum tile -> strided
  copies into vT9 [hw, 9] per head (col 0 = ones; accumulates sum(exp)).
  PE warmup: fp32 junk matmuls anchored on the last x chunk keep the HAM
  clock at 2.4 GHz before the attention stream begins.
  Attention per head: 4 i-windows of 1024; for each of 32 j-tiles: QK
  matmuls -> psum fp32 [128(j), 1024(i)] (3-deep rotation so the PE never
  stalls on the exp engines).  exp is split across BOTH engines: ACT does
  head0's tile (+ first EXSPLIT cols of head1) with the exact Exp LUT;
  DVE does the rest of head1 via a Schraudolph bit-trick (one
  tensor_scalar mult+add writing round-to-nearest int16 that IS the bf16
  bit pattern of exp(s); constant-factor bias cancels in softmax).
  AV matmuls (one j-tile behind) accumulate [9,512] blocks at psum
  partitions 32g over all 32 j-tiles.
  Unnormalized outputs + sumexp go through ONE AllToAll (each core
  receives only its own 512-column slice: 288KB instead of 2.3MB), then a
  per-core sliced tail: recip (fast approx) -> f-broadcast via DRAM
  bounce -> multiply -> block-diag wp matmul -> fused +bias+residual.
  Each core outputs its [128, 512] column slice; host concatenates.
"""
import sys

import numpy as np

sys.path.insert(0, "/opt/trn_rl_repo")

T, C, HW, NCORES = 8, 16, 4096, 8
N_ELEM = T * HW  # per-channel element count for BN stats
EPS = 1e-5
SCALE = float(T) ** -0.5
WWIN = 1024                     # i-window width
NWIN = HW // WWIN               # 4 windows
NJT = 32                        # j-tiles of 128
CHK = 1024                      # x load / stats chunk
EXSPLIT = 64                    # cols of head1's exp done by ACT (balance)
# Schraudolph bf16 exp: bits = trunc(s * A + B); bitcast int16 -> bf16.
# +0.5 turns the fp32->int16 truncation into round-to-nearest; c tuned to
# center the sawtooth rel-error (any constant factor cancels in softmax).
EXP_A = 128.0 / float(np.log(2.0))
EXP_C = 11.75
EXP_B = 127.0 * 128.0 - EXP_C + 0.5

_CACHE = {}


def _build_program():
    import concourse.bass as bass
    import concourse.bacc as bacc
    import concourse.tile as tile
    from concourse import mybir

    f32 = mybir.dt.float32
    bf16 = mybir.dt.bfloat16
    i16 = mybir.dt.int16
    AX = mybir.AxisListType
    OP = mybir.AluOpType
    ACT = mybir.ActivationFunctionType

    nc = bacc.Bacc("TRN2", target_bir_lowering=False, debug=False,
                   num_devices=NCORES)
    x = nc.dram_tensor("x", [128, HW], f32, kind="ExternalInput").ap()
    x_res = nc.dram_tensor("x_res", [128, 512], f32,
                           kind="ExternalInput").ap()
    wqk_bd = nc.dram_tensor("wqk_bd", [128, 128], bf16,
                            kind="ExternalInput").ap()
    bqk_col = nc.dram_tensor("bqk_col", [128, 1], f32,
                             kind="ExternalInput").ap()
    wv_rhs = nc.dram_tensor("wv_rhs", [128, 18], bf16,
                            kind="ExternalInput").ap()
    wp_bd = nc.dram_tensor("wp_bd", [128, 128], bf16,
                           kind="ExternalInput").ap()
    bp_col = nc.dram_tensor("bp_col", [128, 1], f32,
                            kind="ExternalInput").ap()
    sel = nc.dram_tensor("sel", [128, 16], f32, kind="ExternalInput").ap()
    out = nc.dram_tensor("out", [128, 512], f32, kind="ExternalOutput").ap()

    with tile.TileContext(nc) as tc:
        with (
            tc.tile_pool(name="persist", bufs=1) as P1,
            tc.tile_pool(name="work", bufs=4) as PW,
            tc.tile_pool(name="scratch", bufs=1) as PS,
            tc.tile_pool(name="psq", bufs=3, space="PSUM") as PQK,
            tc.tile_pool(name="psa", bufs=2, space="PSUM") as PAV,
            tc.tile_pool(name="dram", bufs=1, space="DRAM") as PD,
        ):
            # ---------------- loads ----------------
            x_sb = P1.tile([128, HW], f32)
            for ch in range(HW // CHK):
                nc.sync.dma_start(out=x_sb[:, ch * CHK:(ch + 1) * CHK],
                                  in_=x[:, ch * CHK:(ch + 1) * CHK])
            xres_sb = P1.tile([128, 512], f32)
            nc.sync.dma_start(out=xres_sb, in_=x_res)
            wqk_sb = P1.tile([128, 128], bf16)
            nc.sync.dma_start(out=wqk_sb, in_=wqk_bd)
            bqk_sb = P1.tile([128, 1], f32)
            nc.sync.dma_start(out=bqk_sb, in_=bqk_col)
            wvrhs_sb = P1.tile([128, 18], bf16)
            nc.sync.dma_start(out=wvrhs_sb, in_=wv_rhs)
            wpbd_sb = P1.tile([128, 128], bf16)
            nc.sync.dma_start(out=wpbd_sb, in_=wp_bd)
            bpcol_sb = P1.tile([128, 1], f32)
            nc.sync.dma_start(out=bpcol_sb, in_=bp_col)
            sel_sb = P1.tile([128, 16], f32)
            nc.sync.dma_start(out=sel_sb, in_=sel)

            # ---------------- BN stats (pipelined per chunk) -------------
            zero128 = P1.tile([128, 1], f32)
            nc.vector.memset(zero128, 0.0)
            eps_t = P1.tile([1, 1], f32)
            nc.vector.memset(eps_t, EPS)
            zero_t = P1.tile([1, 1], f32)
            nc.vector.memset(zero_t, 0.0)
            s1c = P1.tile([128, 4], f32)
            s2c = P1.tile([128, 4], f32)
            sqj = PS.tile([128, CHK], bf16, tag="sqj")
            for ch in range(HW // CHK):
                xc = x_sb[:, ch * CHK:(ch + 1) * CHK]
                nc.vector.reduce_sum(out=s1c[:, ch:ch + 1], in_=xc, axis=AX.X)
                nc.scalar.activation(sqj, xc, ACT.Square, bias=zero128,
                                     accum_out=s2c[:, ch:ch + 1])
            s1 = P1.tile([128, 2], f32)
            nc.vector.reduce_sum(out=s1[:, 0:1], in_=s1c, axis=AX.X)
            nc.vector.reduce_sum(out=s1[:, 1:2], in_=s2c, axis=AX.X)
            ps_st = PAV.tile([1, 32], f32, tag="av")
            nc.tensor.matmul(ps_st[:, 0:16], lhsT=s1[:, 0:1], rhs=sel_sb,
                             start=True, stop=True)
            nc.tensor.matmul(ps_st[:, 16:32], lhsT=s1[:, 1:2], rhs=sel_sb,
                             start=True, stop=True)
            stats = P1.tile([1, 32], f32)
            nc.vector.tensor_scalar_mul(stats, ps_st, 1.0 / N_ELEM)
            var = P1.tile([1, 16], f32)
            nc.vector.tensor_mul(var, stats[:, 0:16], stats[:, 0:16])
            nc.vector.tensor_sub(var, stats[:, 16:32], var)
            inv = P1.tile([1, 16], f32)
            nc.scalar.activation(inv, var, ACT.Ln, bias=eps_t)
            nc.scalar.activation(inv, inv, ACT.Exp, scale=-0.5, bias=zero_t)
            # bounce mean/inv through DRAM to broadcast [1,16] -> [128,1]
            st_dram = PD.tile([2, 16], f32)
            nc.sync.dma_start(out=st_dram[0:1, :], in_=stats[:, 0:16])
            nc.sync.dma_start(out=st_dram[1:2, :], in_=inv)
            mean_p = P1.tile([128, 1], f32)
            inv_p = P1.tile([128, 1], f32)
            for dst, row in ((mean_p, st_dram[0:1, :]),
                             (inv_p, st_dram[1:2, :])):
                src = bass.AP(tensor=row.tensor, offset=row.offset,
                              ap=[[0, T], list(row.ap[-1])])
                nc.gpsimd.dma_start(out=dst[:], in_=src)

            # ---------------- PE warmup (HAM -> 2.4 GHz) -----------------
            # fp32 junk matmuls anchored on the LAST x chunk: ~5us of dense
            # PE work ending right as xhat/projections begin, so the HAM
            # SHORT window sees >=3.4us sustained busy and un-throttles.
            for wi in range(6):
                pj = PQK.tile([128, 512], f32, tag="qk", name=f"warm{wi}")
                nc.tensor.matmul(pj, lhsT=x_sb[:, 3072 + wi * 128:
                                               3200 + wi * 128],
                                 rhs=x_sb[:, 3072:3584],
                                 start=True, stop=True)

            # ---------------- xhat (2 halves) ----------------------------
            xhat = P1.tile([128, HW], bf16)
            for h in range(2):
                sl = slice(h * 2048, (h + 1) * 2048)
                nc.vector.tensor_scalar(out=xhat[:, sl], in0=x_sb[:, sl],
                                        scalar1=mean_p, scalar2=inv_p,
                                        op0=OP.subtract, op1=OP.mult)

            # ---------------- q & k projection (fused, bf16) -------------
            q_sb = P1.tile([64, HW], bf16)
            k_sb = P1.tile([64, HW], bf16)
            for ch in range(HW // 1024):
                pp = PQK.tile([128, 1024], f32, tag="qk", name=f"proj{ch}")
                for b in range(2):
                    nc.tensor.matmul(pp[:, b * 512:(b + 1) * 512],
                                     lhsT=wqk_sb,
                                     rhs=xhat[:, ch * 1024 + b * 512:
                                              ch * 1024 + (b + 1) * 512],
                                     start=True, stop=True)
                csl = slice(ch * 1024, (ch + 1) * 1024)
                nc.vector.tensor_scalar_add(out=q_sb[:, csl],
                                            in0=pp[0:64, :],
                                            scalar1=bqk_sb[0:64])
                nc.scalar.activation(k_sb[:, csl], pp[64:128, :],
                                     ACT.Identity, bias=bqk_sb[64:128])

            # ---------------- v -> vT9 per head (bf16, ones col 0) -------
            vT9 = []
            for l in range(2):
                t9 = P1.tile([128, 32, 9], bf16, tag=f"t9_{l}")
                nc.vector.memset(t9[:, :, 0:1], 1.0)
                vT9.append(t9)
            for g in range(4):
                vps = PAV.tile([128, 144], f32, tag="av", name=f"vps{g}")
                for jj in range(8):
                    jc = g * 8 + jj
                    nc.tensor.matmul(vps[:, jj * 18:(jj + 1) * 18],
                                     lhsT=xhat[:, jc * 128:(jc + 1) * 128],
                                     rhs=wvrhs_sb, start=True, stop=True)
                vv = vps[:]
                for l in range(2):
                    src = bass.AP(tensor=vv.tensor,
                                  offset=vv.offset + l * 9 + 1,
                                  ap=[list(vv.ap[0]), [18, 8], [1, 8]])
                    dst = vT9[l][:, g * 8:(g + 1) * 8, 1:9]
                    if l == 0:
                        nc.scalar.activation(dst, src, ACT.Copy)
                    else:
                        nc.vector.tensor_copy(dst, src)

            # ---------------- attention ----------------
            cc_in = nc.dram_tensor("cc_in", [NCORES, 18 * 512], f32).ap()
            for w in range(NWIN):
                i0 = w * WWIN
                avs = [PAV.tile([128, 512], f32, tag="av",
                                name=f"av_{l}_{w}") for l in range(2)]
                # AV runs one j-tile behind QK/exp.
                ex_prev = [None, None]
                for jt in range(NJT + 1):
                    ex_cur = [None, None]
                    if jt < NJT:
                        qks = [PQK.tile([128, WWIN], f32, tag="qk",
                                        name=f"qk_{l}") for l in range(2)]
                        for b in range(WWIN // 512):
                            for l in range(2):
                                nc.tensor.matmul(
                                    qks[l][:, b * 512:(b + 1) * 512],
                                    lhsT=k_sb[l * 32:l * 32 + 8,
                                              jt * 128:(jt + 1) * 128],
                                    rhs=q_sb[l * 32:l * 32 + 8,
                                             i0 + b * 512:
                                             i0 + (b + 1) * 512],
                                    start=True, stop=True)
                        # exp: ACT head0 (+ first EXSPLIT of head1, exact),
                        # DVE rest of head1 (Schraudolph int16 == bf16 bits)
                        ex0 = PW.tile([128, WWIN], bf16, tag="ex")
                        nc.scalar.activation(ex0, qks[0], ACT.Exp, bias=zero128)
                        ex1 = PW.tile([128, WWIN], bf16, tag="ex")
                        nc.scalar.activation(ex1[:, 0:EXSPLIT],
                                             qks[1][:, 0:EXSPLIT], ACT.Exp,
                                             bias=zero128)
                        nc.vector.tensor_scalar(
                            out=ex1[:, EXSPLIT:WWIN].bitcast(i16),
                            in0=qks[1][:, EXSPLIT:WWIN],
                            scalar1=float(EXP_A), scalar2=float(EXP_B),
                            op0=OP.mult, op1=OP.add)
                        ex_cur = [ex0, ex1]
                    if jt > 0:
                        for l in range(2):
                            for g in range(WWIN // 512):
                                nc.tensor.matmul(
                                    avs[l][32 * g:32 * g + 9, :],
                                    lhsT=vT9[l][:, jt - 1, :],
                                    rhs=ex_prev[l][:, g * 512:(g + 1) * 512],
                                    start=(jt == 1), stop=(jt == NJT),
                                    tile_position=(0, 32 * g),
                                    skip_group_check=True)
                    ex_prev = ex_cur
                # copy psum -> sbuf, ship unnormalized rows + sumexp to the
                # peer owning each 512-column chunk (AllToAll layout).
                for l in range(2):
                    s128 = PW.tile([128, 512], f32, tag="s128")
                    if l == 0:
                        nc.scalar.activation(s128, avs[l], ACT.Copy)
                    else:
                        nc.vector.tensor_copy(s128, avs[l])
                    for g in range(WWIN // 512):
                        p = 2 * w + g
                        dst = bass.AP(tensor=cc_in.tensor,
                                      offset=p * 9216 + l * 4608,
                                      ap=[[512, 9], [1, 512]])
                        nc.sync.dma_start(out=dst,
                                          in_=s128[32 * g:32 * g + 9, :])

            # ------------- AllToAll + sliced normalize + output proj -----
            cc_out = nc.dram_tensor("cc_out", [NCORES, 18 * 512], f32).ap()
            nc.gpsimd.collective_compute(
                "AllToAll", OP.bypass,
                replica_groups=[list(range(NCORES))],
                ins=[cc_in.opt()], outs=[cc_out.opt()])
            # sumexp rows of all 16 heads for MY column slice -> recip
            rsum = P1.tile([16, 512], f32)
            src = bass.AP(tensor=cc_out.tensor, offset=0,
                          ap=[[9216, 8], [4608, 2], [1, 512]])
            nc.sync.dma_start(out=rsum[:], in_=src)
            rinv = P1.tile([16, 512], f32)
            nc.vector.reciprocal_approx_fast(rinv, rsum)
            rdram = PD.tile([16, 512], f32)
            nc.sync.dma_start(out=rdram[:], in_=rinv[:])
            rbc = P1.tile([128, 512], f32)
            rd_t = rdram[:].tensor
            src2 = bass.AP(tensor=rd_t, offset=rdram[:].offset,
                           ap=[[512, 16], [0, T], [1, 512]])
            nc.sync.dma_start(out=rbc[:], in_=src2)
            # unnormalized attention rows (c,f layout) for MY columns
            acf = P1.tile([128, 512], f32)
            src3 = bass.AP(tensor=cc_out.tensor, offset=512,
                           ap=[[9216, 8], [4608, 2], [512, 8], [1, 512]])
            nc.sync.dma_start(out=acf[:], in_=src3)
            att_n = P1.tile([128, 512], bf16)
            nc.vector.tensor_mul(att_n, acf, rbc)
            psp = PQK.tile([128, 512], f32, tag="qk", name="pconv")
            nc.tensor.matmul(psp, lhsT=wpbd_sb, rhs=att_n,
                             start=True, stop=True)
            och = P1.tile([128, 512], f32)
            nc.vector.scalar_tensor_tensor(
                out=och, in0=psp, scalar=bpcol_sb,
                in1=xres_sb, op0=OP.add, op1=OP.add)
            nc.sync.dma_start(out=out, in_=och)

    nc.compile()
    return nc


def host_inputs(r, x128, gamma, beta, wq, bq, wk, bk, wv, bv, wp, bp):
    """Per-core host-side input prep (folds gamma/beta/biases/scale)."""
    import ml_dtypes
    bf = ml_dtypes.bfloat16
    wq_e = (wq * gamma[None, :] * SCALE).astype(np.float32)
    wk_e = (wk * gamma[None, :]).astype(np.float32)
    wv_e = (wv * gamma[None, :]).astype(np.float32)
    bq_e = ((bq + wq @ beta) * SCALE).astype(np.float32)
    bk_e = (bk + wk @ beta).astype(np.float32)
    bv_e = (bv + wv @ beta).astype(np.float32)
    bp_e = (bp + wp @ bv_e).astype(np.float32)

    wqk_bd = np.zeros((128, 128), np.float32)
    bqk_col = np.zeros((128, 1), np.float32)
    wv_rhs = np.zeros((128, 18), np.float32)
    fi = np.arange(T)
    ci = np.arange(C)
    for l in range(2):
        n = 2 * r + l
        rows = fi[:, None] * 16 + ci[None, :]
        wqk_bd[rows, (l * 32 + fi)[:, None]] = wq_e[n]
        wqk_bd[rows, (64 + l * 32 + fi)[:, None]] = wk_e[n]
        wv_rhs[rows, (l * 9 + 1 + fi)[:, None]] = wv_e[n]
        bqk_col[l * 32 + fi, 0] = bq_e[n]
        bqk_col[64 + l * 32 + fi, 0] = bk_e[n]
    # p-conv lhsT rows are in (c,f) order to match the gathered layout
    wp_bd = np.zeros((128, 128), np.float32)
    bp_col = np.zeros((128, 1), np.float32)
    for f in range(T):
        wp_bd[np.ix_(ci * 8 + f, f * 16 + ci)] = wp.T
        bp_col[f * 16 + ci, 0] = bp_e
    selm = np.zeros((128, 16), np.float32)
    selm[np.arange(128), np.tile(ci, T)] = 1.0
    x_res = np.ascontiguousarray(x128[:, r * 512:(r + 1) * 512])
    return dict(x=x128, x_res=x_res, wqk_bd=wqk_bd.astype(bf),
                bqk_col=bqk_col, wv_rhs=wv_rhs.astype(bf),
                wp_bd=wp_bd.astype(bf), bp_col=bp_col, sel=selm)


def make_in_maps(inputs):
    x = np.ascontiguousarray(np.asarray(inputs["x"], np.float32))
    x128 = x.reshape(128, HW)
    args = {k: np.asarray(v, np.float32) for k, v in inputs.items()
            if k != "x"}
    return [host_inputs(r, x128, **args) for r in range(NCORES)]


def run(inputs, trace=False):
    """Returns (out (8,16,64,64) f32, BassKernelResults)."""
    from concourse.bass_utils import run_bass_kernel_spmd
    if "nc" not in _CACHE:
        _CACHE["nc"] = _build_program()
    nc = _CACHE["nc"]
    in_maps = make_in_maps(inputs)
    res = run_bass_kernel_spmd(nc, in_maps, list(range(NCORES)), trace=trace)
    out128 = np.empty((128, HW), np.float32)
    for r in range(NCORES):
        out128[:, r * 512:(r + 1) * 512] = np.asarray(
            res.results[r]["out"], np.float32)
    return out128.reshape(T, C, 64, 64), res


def kernel(**inputs):
    out, _ = run(inputs, trace=False)
    return out
